# revision 8
# baseline (speedup 1.0000x reference)
"""Chamfer loss Bass/Tile kernel for Trainium2 (8 NeuronCores, SPMD).

Problem: x, y [B=32, D=128, N=2048] f32, mask [B, N] bool (shared by x and y).
  d[b,i,j] = ||x_i - y_j||^2;  loss = mean_b( sum_j min_i d + sum_i min_j d )
  (mins/sums over valid entries only).

Strategy (data-parallel over batch, 4 batches per core):
  - Work in S = -d/2 = G - x2/2 - y2/2 form. PSUM group [128 x 1024] gets
    S directly: an fp8e4 DoubleRow "prefill" matmul (K=2, hi/lo split rows,
    0.5 cycles/col) injects -x2m/2 (per-partition) and -y2m/2 (per-column)
    including +BIG masking, then bf16 main matmuls accumulate G on top.
    No per-tile bias work remains for the vector engines.
  - Evac+col: ~70% of groups evacuate PSUM->bf16 SBUF via ACT (Copy) and
    run a DVE tensor_scalar (4x mode) whose accum_out gives max_j S per
    partition; ~30% of groups use a single Pool tensor_scalar that does
    evac + accum in one op. min_j d = -2 max_j S.
  - Row path: running elementwise max over the 16 i-chunks (TT max, split
    DVE/Pool); the ic==0 evac writes straight into R. Finished per batch by
    PE transposes + a grouped tensor_reduce -> max_i S per j.
  - Masked sums against host-shipped mask cols; host scales by -2/B.
"""

import numpy as np
import ml_dtypes
from contextlib import ExitStack

import concourse.mybir as mybir
import concourse.tile as tile
from concourse import bacc
from concourse.masks import make_identity

F32 = mybir.dt.float32
BF16 = mybir.dt.bfloat16
F8E4 = mybir.dt.float8e4
AX = mybir.AxisListType
OP = mybir.AluOpType
ACTF = mybir.ActivationFunctionType
DR = mybir.MatmulPerfMode.DoubleRow

B, D, N = 32, 128, 2048
CORES = 8
BPC = B // CORES          # batches per core
ICH, NI = 128, N // 128   # i-chunk size / count
GW, NG = 1024, N // 1024  # j-group width / count (evac granularity)
MMW = 512                 # matmul width (one PSUM bank)
MBIG = 288.0              # mask push (d shifted by 2*MBIG per masked side)

# strips (ic) that evac via Pool tensor_scalar (fused col accum); the rest
# evac via ACT Copy + a strip-wide DVE col pass.
POOL_EVAC = frozenset((3, 7, 10, 13))
# row TT ops (ic>=1): sent to Pool for these strips, else DVE.
ROW_POOL = frozenset((5, 9, 14))


def build_nc():
    nc = bacc.Bacc("TRN2", target_bir_lowering=False, debug=False)
    x_d = nc.dram_tensor("x", [BPC, D, N], BF16, kind="ExternalInput").ap()
    y_d = nc.dram_tensor("y", [BPC, D, N], BF16, kind="ExternalInput").ap()
    pfl_d = nc.dram_tensor("pfl", [BPC, 2, 2, N], F8E4, kind="ExternalInput").ap()
    pfr_d = nc.dram_tensor("pfr", [BPC, 2, 2, N], F8E4, kind="ExternalInput").ap()
    mcols_d = nc.dram_tensor("mcols", [BPC, D, NI], F32, kind="ExternalInput").ap()
    out_d = nc.dram_tensor("out", [D, 2], F32, kind="ExternalOutput").ap()

    with tile.TileContext(nc) as tc:
        with ExitStack() as ctx:
            _emit(ctx, tc, out_d, x_d, y_d, pfl_d, pfr_d, mcols_d)
    nc.compile()
    return nc


def _emit(ctx, tc, out_d, x_d, y_d, pfl_d, pfr_d, mcols_d):
    nc = tc.nc
    io = ctx.enter_context(tc.tile_pool(name="io", bufs=2))
    pf = ctx.enter_context(tc.tile_pool(name="pf", bufs=2))
    bp = ctx.enter_context(tc.tile_pool(name="bp", bufs=6))
    rp = ctx.enter_context(tc.tile_pool(name="rp", bufs=2))
    scp = ctx.enter_context(tc.tile_pool(name="scp", bufs=2))
    small = ctx.enter_context(tc.tile_pool(name="small", bufs=2))
    accp = ctx.enter_context(tc.tile_pool(name="accp", bufs=1))
    pre = ctx.enter_context(tc.tile_pool(name="pre", bufs=1))
    pp = ctx.enter_context(tc.tile_pool(name="pp", bufs=3, space="PSUM"))
    prt = ctx.enter_context(tc.tile_pool(name="prt", bufs=1, space="PSUM"))

    acc = accp.tile([D, 2], F32)
    nc.vector.memset(acc[:], 0.0)
    ident = pre.tile([ICH, ICH], BF16, tag="ident")
    make_identity(nc, ident[:])

    def emit_load(b):
        st = {}
        st["xs"] = io.tile([D, N], BF16, tag="xs", name=f"xs{b}")
        st["ys"] = io.tile([D, N], BF16, tag="ys", name=f"ys{b}")
        st["pfl"] = pf.tile([2, 2, N], F8E4, tag="pfl", name=f"pfl{b}")
        st["pfr"] = pf.tile([2, 2, N], F8E4, tag="pfr", name=f"pfr{b}")
        st["mcols"] = small.tile([D, NI], F32, tag="mcols", name=f"mcols{b}")
        nc.sync.dma_start(out=st["pfl"][:], in_=pfl_d[b])
        nc.sync.dma_start(out=st["pfr"][:], in_=pfr_d[b])
        nc.sync.dma_start(out=st["mcols"][:], in_=mcols_d[b])
        nc.sync.dma_start(out=st["xs"][:], in_=x_d[b])
        nc.sync.dma_start(out=st["ys"][:], in_=y_d[b])
        return st

    st = emit_load(0)
    nxt = None
    for b in range(BPC):
        xs, ys, pfl, pfr, mcols = (st["xs"], st["ys"], st["pfl"], st["pfr"],
                                   st["mcols"])
        R = rp.tile([D, N], BF16, tag="R", name=f"R{b}")
        cm = small.tile([D, NI * NG], F32, tag="cm", name=f"cm{b}")
        nc.vector.memset(cm[:], -1e30)
        for ic in range(NI):
            lsl = slice(ic * ICH, (ic + 1) * ICH)
            for jg in range(NG):
                gi = ic * NG + jg
                ps = pp.tile([D, GW], F32, tag="ps")
                for h in range(GW // MMW):
                    j0 = jg * GW + h * MMW
                    psl = ps[:, h * MMW:(h + 1) * MMW]
                    nc.tensor.matmul(psl, lhsT=pfl[:, :, lsl],
                                     rhs=pfr[:, :, j0:j0 + MMW],
                                     start=True, stop=False, perf_mode=DR)
                    nc.tensor.matmul(psl, lhsT=xs[:, lsl],
                                     rhs=ys[:, j0:j0 + MMW],
                                     start=False, stop=True)
                rsl = R[:, jg * GW:(jg + 1) * GW]
                bdst = rsl if ic == 0 else bp.tile([D, GW], BF16, tag="bt")
                if ic in POOL_EVAC:
                    nc.gpsimd.tensor_scalar(bdst, ps[:], 0.0, None,
                                            op0=OP.add, op1=OP.max,
                                            accum_out=cm[:, gi:gi + 1])
                else:
                    nc.scalar.activation(bdst, ps[:], ACTF.Copy,
                                         bias=0.0, scale=1.0)
                    scr = scp.tile([D, GW], BF16, tag="scr")
                    nc.vector.tensor_scalar(scr[:], bdst, 0.0, None,
                                            op0=OP.add, op1=OP.max,
                                            accum_out=cm[:, gi:gi + 1])
                if ic > 0:
                    eng = nc.gpsimd if ic in ROW_POOL else nc.vector
                    eng.tensor_tensor(rsl, bdst, rsl, op=OP.max)
            if ic == 2 and b + 1 < BPC:
                nxt = emit_load(b + 1)

        # row finish: max over partitions via PE transposes + grouped reduce
        rt = prt.tile([D, N], BF16, tag="rt")
        for t in range(NI):
            nc.tensor.transpose(rt[:, t * ICH:(t + 1) * ICH],
                                R[:, t * ICH:(t + 1) * ICH], ident[:])
        rr = small.tile([D, NI], F32, tag="rr")
        nc.vector.tensor_reduce(rr[:], rt[:].rearrange("p (t q) -> p t q", q=ICH),
                                axis=AX.X, op=OP.max)
        cmf = small.tile([D, NI], F32, tag="cmf")
        nc.vector.tensor_reduce(cmf[:], cm[:].rearrange("p (i g) -> p i g", g=NG),
                                axis=AX.X, op=OP.max)
        tX = small.tile([D, NI], F32, tag="tX")
        nc.vector.tensor_tensor(tX[:], rr[:], mcols[:], op=OP.mult)
        tY = small.tile([D, NI], F32, tag="tY")
        nc.vector.tensor_tensor(tY[:], cmf[:], mcols[:], op=OP.mult)
        sX = small.tile([D, 1], F32, tag="sX")
        nc.vector.tensor_reduce(sX[:], tX[:], axis=AX.X, op=OP.add)
        sY = small.tile([D, 1], F32, tag="sY")
        nc.vector.tensor_reduce(sY[:], tY[:], axis=AX.X, op=OP.add)
        nc.vector.tensor_tensor(acc[:, 0:1], acc[:, 0:1], sX[:], op=OP.add)
        nc.vector.tensor_tensor(acc[:, 1:2], acc[:, 1:2], sY[:], op=OP.add)
        if nxt is not None:
            st = nxt
            nxt = None

    nc.sync.dma_start(out=out_d, in_=acc[:])


def _hilo_e4m3(v):
    """Split v >= 0 into hi+lo fp8e4m3 (clipped to the 240 max)."""
    hi = np.minimum(v, 240.0).astype(ml_dtypes.float8_e4m3)
    lo = (v - hi.astype(np.float64)).astype(ml_dtypes.float8_e4m3)
    return hi, lo


def prepare_in_maps(x, y, mask):
    xb = np.asarray(x).astype(ml_dtypes.bfloat16)          # [B, D, N]
    yb = np.asarray(y).astype(ml_dtypes.bfloat16)
    mf = np.asarray(mask).astype(np.float64)               # [B, N]
    x2 = (xb.astype(np.float64) ** 2).sum(axis=1)          # [B, N]
    y2 = (yb.astype(np.float64) ** 2).sum(axis=1)
    vx = x2 / 2 + MBIG * (1.0 - mf)
    vy = y2 / 2 + MBIG * (1.0 - mf)
    xhi, xlo = _hilo_e4m3(vx)
    yhi, ylo = _hilo_e4m3(vy)
    pfl = np.empty((B, 2, 2, N), dtype=ml_dtypes.float8_e4m3)
    pfr = np.empty((B, 2, 2, N), dtype=ml_dtypes.float8_e4m3)
    pfl[:, 0, :, :] = -1.0
    pfl[:, 1, 0, :] = xhi
    pfl[:, 1, 1, :] = xlo
    pfr[:, 0, 0, :] = yhi
    pfr[:, 0, 1, :] = ylo
    pfr[:, 1, :, :] = -1.0
    mcols = np.ascontiguousarray(
        mf.astype(np.float32).reshape(B, NI, ICH).transpose(0, 2, 1))
    in_maps = []
    for c in range(CORES):
        s = slice(c * BPC, (c + 1) * BPC)
        in_maps.append({
            "x": np.ascontiguousarray(xb[s]),
            "y": np.ascontiguousarray(yb[s]),
            "pfl": np.ascontiguousarray(pfl[s]),
            "pfr": np.ascontiguousarray(pfr[s]),
            "mcols": np.ascontiguousarray(mcols[s]),
        })
    return in_maps


def finish(per_core_outs):
    """per_core_outs: list of 8 arrays [128, 2] -> scalar loss."""
    total = 0.0
    for o in per_core_outs:
        total += np.asarray(o, dtype=np.float64).sum()
    return np.float32(-2.0 * total / B)


_NC = None


def kernel(x, y, mask):
    global _NC
    if _NC is None:
        _NC = build_nc()
    from concourse.bass_utils import run_bass_kernel_spmd
    in_maps = prepare_in_maps(np.asarray(x), np.asarray(y), np.asarray(mask))
    res = run_bass_kernel_spmd(_NC, in_maps, list(range(CORES)))
    return finish([res.results[c]["out"] for c in range(CORES)])


# revision 10
# speedup vs baseline: 1.1077x; 1.1077x over previous
"""Chamfer loss Bass/Tile kernel for Trainium2 (8 NeuronCores, SPMD).

Problem: x, y [B=32, D=128, N=2048] f32, mask [B, N] bool (shared by x and y).
  d[b,i,j] = ||x_i - y_j||^2;  loss = mean_b( sum_j min_i d + sum_i min_j d )
  (mins/sums over valid entries only).

Strategy (data-parallel over batch, 4 batches per core):
  - Work in S = -d/2 = G - x2/2 - y2/2 form. PSUM group [128 x 1024] gets
    S directly: an fp8e4 DoubleRow "prefill" matmul (K=2, hi/lo split rows,
    0.5 cycles/col) injects -x2m/2 (per-partition) and -y2m/2 (per-column)
    including +BIG masking, then bf16 main matmuls accumulate G on top.
    No per-tile bias work remains for the vector engines.
  - Evac+col: ~70% of groups evacuate PSUM->bf16 SBUF via ACT (Copy) and
    run a DVE tensor_scalar (4x mode) whose accum_out gives max_j S per
    partition; ~30% of groups use a single Pool tensor_scalar that does
    evac + accum in one op. min_j d = -2 max_j S.
  - Row path: running elementwise max over the 16 i-chunks (TT max, split
    DVE/Pool); the ic==0 evac writes straight into R. Finished per batch by
    PE transposes + a grouped tensor_reduce -> max_i S per j.
  - Masked sums against host-shipped mask cols; host scales by -2/B.
"""

import numpy as np
import ml_dtypes
from contextlib import ExitStack

import concourse.mybir as mybir
import concourse.tile as tile
from concourse import bacc
from concourse.masks import make_identity

F32 = mybir.dt.float32
BF16 = mybir.dt.bfloat16
F8E4 = mybir.dt.float8e4
AX = mybir.AxisListType
OP = mybir.AluOpType
ACTF = mybir.ActivationFunctionType
DR = mybir.MatmulPerfMode.DoubleRow

B, D, N = 32, 128, 2048
CORES = 8
BPC = B // CORES          # batches per core
ICH, NI = 128, N // 128   # i-chunk size / count
GW, NG = 1024, N // 1024  # j-group width / count (evac granularity)
MMW = 512                 # matmul width (one PSUM bank)
MBIG = 288.0              # mask push (d shifted by 2*MBIG per masked side)

# groups (gi = ic*NG+jg) that evac via Pool tensor_scalar (fused col accum);
# the rest evac via ACT Copy + a DVE col pass.
POOL_EVAC = frozenset(gi for gi in range(NI * NG) if gi % 10 in (3, 6, 9))
# row TT ops (ic>=1): sent to Pool when gi % 7 == 2, else DVE.
ROW_POOL_GI = frozenset(gi for gi in range(NI * NG) if gi % 7 == 2)


def build_nc():
    nc = bacc.Bacc("TRN2", target_bir_lowering=False, debug=False)
    x_d = nc.dram_tensor("x", [BPC, D, N], BF16, kind="ExternalInput").ap()
    y_d = nc.dram_tensor("y", [BPC, D, N], BF16, kind="ExternalInput").ap()
    pfl_d = nc.dram_tensor("pfl", [BPC, 2, 2, N], F8E4, kind="ExternalInput").ap()
    pfr_d = nc.dram_tensor("pfr", [BPC, 2, 2, N], F8E4, kind="ExternalInput").ap()
    mcols_d = nc.dram_tensor("mcols", [BPC, D, NI], F32, kind="ExternalInput").ap()
    out_d = nc.dram_tensor("out", [D, 2], F32, kind="ExternalOutput").ap()

    with tile.TileContext(nc) as tc:
        with ExitStack() as ctx:
            _emit(ctx, tc, out_d, x_d, y_d, pfl_d, pfr_d, mcols_d)
    nc.compile()
    return nc


def _emit(ctx, tc, out_d, x_d, y_d, pfl_d, pfr_d, mcols_d):
    nc = tc.nc
    io = ctx.enter_context(tc.tile_pool(name="io", bufs=2))
    pf = ctx.enter_context(tc.tile_pool(name="pf", bufs=2))
    bp = ctx.enter_context(tc.tile_pool(name="bp", bufs=6))
    rp = ctx.enter_context(tc.tile_pool(name="rp", bufs=2))
    scp = ctx.enter_context(tc.tile_pool(name="scp", bufs=2))
    small = ctx.enter_context(tc.tile_pool(name="small", bufs=2))
    accp = ctx.enter_context(tc.tile_pool(name="accp", bufs=1))
    pre = ctx.enter_context(tc.tile_pool(name="pre", bufs=1))
    pp = ctx.enter_context(tc.tile_pool(name="pp", bufs=3, space="PSUM"))
    prt = ctx.enter_context(tc.tile_pool(name="prt", bufs=1, space="PSUM"))

    acc = accp.tile([D, 2], F32)
    nc.vector.memset(acc[:], 0.0)
    ident = pre.tile([ICH, ICH], BF16, tag="ident")
    make_identity(nc, ident[:])

    def emit_load(b):
        st = {}
        st["xs"] = io.tile([D, N], BF16, tag="xs", name=f"xs{b}")
        st["ys"] = io.tile([D, N], BF16, tag="ys", name=f"ys{b}")
        st["pfl"] = pf.tile([2, 2, N], F8E4, tag="pfl", name=f"pfl{b}")
        st["pfr"] = pf.tile([2, 2, N], F8E4, tag="pfr", name=f"pfr{b}")
        st["mcols"] = small.tile([D, NI], F32, tag="mcols", name=f"mcols{b}")
        nc.sync.dma_start(out=st["pfl"][:], in_=pfl_d[b])
        nc.sync.dma_start(out=st["pfr"][:], in_=pfr_d[b])
        nc.sync.dma_start(out=st["mcols"][:], in_=mcols_d[b])
        nc.sync.dma_start(out=st["xs"][:], in_=x_d[b])
        nc.sync.dma_start(out=st["ys"][:], in_=y_d[b])
        return st

    st = emit_load(0)
    nxt = None
    for b in range(BPC):
        xs, ys, pfl, pfr, mcols = (st["xs"], st["ys"], st["pfl"], st["pfr"],
                                   st["mcols"])
        R = rp.tile([D, N], BF16, tag="R", name=f"R{b}")
        cm = small.tile([D, NI * NG], F32, tag="cm", name=f"cm{b}")
        nc.vector.memset(cm[:], -1e30)
        for ic in range(NI):
            lsl = slice(ic * ICH, (ic + 1) * ICH)
            for jg in range(NG):
                gi = ic * NG + jg
                ps = pp.tile([D, GW], F32, tag="ps")
                for h in range(GW // MMW):
                    j0 = jg * GW + h * MMW
                    psl = ps[:, h * MMW:(h + 1) * MMW]
                    nc.tensor.matmul(psl, lhsT=pfl[:, :, lsl],
                                     rhs=pfr[:, :, j0:j0 + MMW],
                                     start=True, stop=False, perf_mode=DR)
                    nc.tensor.matmul(psl, lhsT=xs[:, lsl],
                                     rhs=ys[:, j0:j0 + MMW],
                                     start=False, stop=True)
                rsl = R[:, jg * GW:(jg + 1) * GW]
                bdst = rsl if ic == 0 else bp.tile([D, GW], BF16, tag="bt")
                if gi in POOL_EVAC:
                    nc.gpsimd.tensor_scalar(bdst, ps[:], 0.0, None,
                                            op0=OP.add, op1=OP.max,
                                            accum_out=cm[:, gi:gi + 1])
                else:
                    nc.scalar.activation(bdst, ps[:], ACTF.Copy,
                                         bias=0.0, scale=1.0)
                    scr = scp.tile([D, GW], BF16, tag="scr")
                    nc.vector.tensor_scalar(scr[:], bdst, 0.0, None,
                                            op0=OP.add, op1=OP.max,
                                            accum_out=cm[:, gi:gi + 1])
                if ic > 0:
                    eng = nc.gpsimd if gi in ROW_POOL_GI else nc.vector
                    eng.tensor_tensor(rsl, bdst, rsl, op=OP.max)
            if ic == 2 and b + 1 < BPC:
                nxt = emit_load(b + 1)

        # row finish: max over partitions via PE transposes + grouped reduce
        rt = prt.tile([D, N], BF16, tag="rt")
        for t in range(NI):
            nc.tensor.transpose(rt[:, t * ICH:(t + 1) * ICH],
                                R[:, t * ICH:(t + 1) * ICH], ident[:])
        rr = small.tile([D, NI], F32, tag="rr")
        nc.vector.tensor_reduce(rr[:], rt[:].rearrange("p (t q) -> p t q", q=ICH),
                                axis=AX.X, op=OP.max)
        cmf = small.tile([D, NI], F32, tag="cmf")
        nc.vector.tensor_reduce(cmf[:], cm[:].rearrange("p (i g) -> p i g", g=NG),
                                axis=AX.X, op=OP.max)
        tX = small.tile([D, NI], F32, tag="tX")
        nc.vector.tensor_tensor(tX[:], rr[:], mcols[:], op=OP.mult)
        tY = small.tile([D, NI], F32, tag="tY")
        nc.vector.tensor_tensor(tY[:], cmf[:], mcols[:], op=OP.mult)
        sX = small.tile([D, 1], F32, tag="sX")
        nc.vector.tensor_reduce(sX[:], tX[:], axis=AX.X, op=OP.add)
        sY = small.tile([D, 1], F32, tag="sY")
        nc.vector.tensor_reduce(sY[:], tY[:], axis=AX.X, op=OP.add)
        nc.vector.tensor_tensor(acc[:, 0:1], acc[:, 0:1], sX[:], op=OP.add)
        nc.vector.tensor_tensor(acc[:, 1:2], acc[:, 1:2], sY[:], op=OP.add)
        if nxt is not None:
            st = nxt
            nxt = None

    nc.sync.dma_start(out=out_d, in_=acc[:])


def _hilo_e4m3(v):
    """Split v >= 0 into hi+lo fp8e4m3 (clipped to the 240 max)."""
    hi = np.minimum(v, 240.0).astype(ml_dtypes.float8_e4m3)
    lo = (v - hi.astype(np.float64)).astype(ml_dtypes.float8_e4m3)
    return hi, lo


def prepare_in_maps(x, y, mask):
    xb = np.asarray(x).astype(ml_dtypes.bfloat16)          # [B, D, N]
    yb = np.asarray(y).astype(ml_dtypes.bfloat16)
    mf = np.asarray(mask).astype(np.float64)               # [B, N]
    x2 = (xb.astype(np.float64) ** 2).sum(axis=1)          # [B, N]
    y2 = (yb.astype(np.float64) ** 2).sum(axis=1)
    vx = x2 / 2 + MBIG * (1.0 - mf)
    vy = y2 / 2 + MBIG * (1.0 - mf)
    xhi, xlo = _hilo_e4m3(vx)
    yhi, ylo = _hilo_e4m3(vy)
    pfl = np.empty((B, 2, 2, N), dtype=ml_dtypes.float8_e4m3)
    pfr = np.empty((B, 2, 2, N), dtype=ml_dtypes.float8_e4m3)
    pfl[:, 0, :, :] = -1.0
    pfl[:, 1, 0, :] = xhi
    pfl[:, 1, 1, :] = xlo
    pfr[:, 0, 0, :] = yhi
    pfr[:, 0, 1, :] = ylo
    pfr[:, 1, :, :] = -1.0
    mcols = np.ascontiguousarray(
        mf.astype(np.float32).reshape(B, NI, ICH).transpose(0, 2, 1))
    in_maps = []
    for c in range(CORES):
        s = slice(c * BPC, (c + 1) * BPC)
        in_maps.append({
            "x": np.ascontiguousarray(xb[s]),
            "y": np.ascontiguousarray(yb[s]),
            "pfl": np.ascontiguousarray(pfl[s]),
            "pfr": np.ascontiguousarray(pfr[s]),
            "mcols": np.ascontiguousarray(mcols[s]),
        })
    return in_maps


def finish(per_core_outs):
    """per_core_outs: list of 8 arrays [128, 2] -> scalar loss."""
    total = 0.0
    for o in per_core_outs:
        total += np.asarray(o, dtype=np.float64).sum()
    return np.float32(-2.0 * total / B)


_NC = None


def kernel(x, y, mask):
    global _NC
    if _NC is None:
        _NC = build_nc()
    from concourse.bass_utils import run_bass_kernel_spmd
    in_maps = prepare_in_maps(np.asarray(x), np.asarray(y), np.asarray(mask))
    res = run_bass_kernel_spmd(_NC, in_maps, list(range(CORES)))
    return finish([res.results[c]["out"] for c in range(CORES)])


# revision 11
# speedup vs baseline: 1.1368x; 1.0263x over previous
"""Chamfer loss Bass/Tile kernel for Trainium2 (8 NeuronCores, SPMD).

Problem: x, y [B=32, D=128, N=2048] f32, mask [B, N] bool (shared by x and y).
  d[b,i,j] = ||x_i - y_j||^2;  loss = mean_b( sum_j min_i d + sum_i min_j d )
  (mins/sums over valid entries only).

Strategy (data-parallel over batch, 4 batches per core):
  - Work in S = -d/2 = G - x2/2 - y2/2 form. PSUM group [128 x 1024] gets
    S directly: an fp8e4 DoubleRow "prefill" matmul (K=2, hi/lo split rows,
    0.5 cycles/col) injects -x2m/2 (per-partition) and -y2m/2 (per-column)
    including +BIG masking, then bf16 main matmuls accumulate G on top.
    No per-tile bias work remains for the vector engines.
  - Evac+col: ~70% of groups evacuate PSUM->bf16 SBUF via ACT (Copy) and
    run a DVE tensor_scalar (4x mode) whose accum_out gives max_j S per
    partition; ~30% of groups use a single Pool tensor_scalar that does
    evac + accum in one op. min_j d = -2 max_j S.
  - Row path: running elementwise max over the 16 i-chunks (TT max, split
    DVE/Pool); the ic==0 evac writes straight into R. Finished per batch by
    PE transposes + a grouped tensor_reduce -> max_i S per j.
  - Masked sums against host-shipped mask cols; host scales by -2/B.
"""

import numpy as np
import ml_dtypes
from contextlib import ExitStack

import concourse.mybir as mybir
import concourse.tile as tile
from concourse import bacc
from concourse.masks import make_identity

F32 = mybir.dt.float32
BF16 = mybir.dt.bfloat16
F8E4 = mybir.dt.float8e4
AX = mybir.AxisListType
OP = mybir.AluOpType
ACTF = mybir.ActivationFunctionType
DR = mybir.MatmulPerfMode.DoubleRow

B, D, N = 32, 128, 2048
CORES = 8
BPC = B // CORES          # batches per core
ICH, NI = 128, N // 128   # i-chunk size / count
GW, NG = 1024, N // 1024  # j-group width / count (evac granularity)
MMW = 512                 # matmul width (one PSUM bank)
MBIG = 288.0              # mask push (d shifted by 2*MBIG per masked side)

# groups (gi = ic*NG+jg) that evac via Pool tensor_scalar (fused col accum);
# the rest evac via ACT Copy + a DVE col pass.
POOL_EVAC = frozenset(gi for gi in range(NI * NG) if gi % 10 in (3, 6, 9))
# row TT ops (ic>=1): sent to Pool when gi % 7 == 2, else DVE.
ROW_POOL_GI = frozenset((2, 5, 11, 15, 20, 24, 28))


def build_nc():
    nc = bacc.Bacc("TRN2", target_bir_lowering=False, debug=False)
    x_d = nc.dram_tensor("x", [BPC, D, N], BF16, kind="ExternalInput").ap()
    y_d = nc.dram_tensor("y", [BPC, D, N], BF16, kind="ExternalInput").ap()
    pfl_d = nc.dram_tensor("pfl", [BPC, 2, 2, N], F8E4, kind="ExternalInput").ap()
    pfr_d = nc.dram_tensor("pfr", [BPC, 2, 2, N], F8E4, kind="ExternalInput").ap()
    mcols_d = nc.dram_tensor("mcols", [BPC, D, NI], F32, kind="ExternalInput").ap()
    out_d = nc.dram_tensor("out", [D, 2], F32, kind="ExternalOutput").ap()

    with tile.TileContext(nc) as tc:
        with ExitStack() as ctx:
            _emit(ctx, tc, out_d, x_d, y_d, pfl_d, pfr_d, mcols_d)
    nc.compile()
    return nc


def _emit(ctx, tc, out_d, x_d, y_d, pfl_d, pfr_d, mcols_d):
    nc = tc.nc
    io = ctx.enter_context(tc.tile_pool(name="io", bufs=2))
    pf = ctx.enter_context(tc.tile_pool(name="pf", bufs=2))
    bp = ctx.enter_context(tc.tile_pool(name="bp", bufs=6))
    rp = ctx.enter_context(tc.tile_pool(name="rp", bufs=2))
    scp = ctx.enter_context(tc.tile_pool(name="scp", bufs=2))
    small = ctx.enter_context(tc.tile_pool(name="small", bufs=2))
    accp = ctx.enter_context(tc.tile_pool(name="accp", bufs=1))
    pre = ctx.enter_context(tc.tile_pool(name="pre", bufs=1))
    pp = ctx.enter_context(tc.tile_pool(name="pp", bufs=3, space="PSUM"))
    prt = ctx.enter_context(tc.tile_pool(name="prt", bufs=1, space="PSUM"))

    acc = accp.tile([D, 2], F32)
    nc.vector.memset(acc[:], 0.0)
    ident = pre.tile([ICH, ICH], BF16, tag="ident")
    make_identity(nc, ident[:])

    def emit_load(b):
        st = {}
        st["xs"] = io.tile([D, N], BF16, tag="xs", name=f"xs{b}")
        st["ys"] = io.tile([D, N], BF16, tag="ys", name=f"ys{b}")
        st["pfl"] = pf.tile([2, 2, N], F8E4, tag="pfl", name=f"pfl{b}")
        st["pfr"] = pf.tile([2, 2, N], F8E4, tag="pfr", name=f"pfr{b}")
        st["mcols"] = small.tile([D, NI], F32, tag="mcols", name=f"mcols{b}")
        nc.sync.dma_start(out=st["pfl"][:], in_=pfl_d[b])
        nc.sync.dma_start(out=st["pfr"][:], in_=pfr_d[b])
        nc.sync.dma_start(out=st["mcols"][:], in_=mcols_d[b])
        nc.sync.dma_start(out=st["xs"][:], in_=x_d[b])
        nc.sync.dma_start(out=st["ys"][:], in_=y_d[b])
        return st

    st = emit_load(0)
    nxt = None
    for b in range(BPC):
        xs, ys, pfl, pfr, mcols = (st["xs"], st["ys"], st["pfl"], st["pfr"],
                                   st["mcols"])
        R = rp.tile([D, N], BF16, tag="R", name=f"R{b}")
        cm = small.tile([D, NI * NG], F32, tag="cm", name=f"cm{b}")
        nc.vector.memset(cm[:], -1e30)
        for ic in range(NI):
            lsl = slice(ic * ICH, (ic + 1) * ICH)
            for jg in range(NG):
                gi = ic * NG + jg
                ps = pp.tile([D, GW], F32, tag="ps")
                for h in range(GW // MMW):
                    j0 = jg * GW + h * MMW
                    psl = ps[:, h * MMW:(h + 1) * MMW]
                    nc.tensor.matmul(psl, lhsT=pfl[:, :, lsl],
                                     rhs=pfr[:, :, j0:j0 + MMW],
                                     start=True, stop=False, perf_mode=DR)
                    nc.tensor.matmul(psl, lhsT=xs[:, lsl],
                                     rhs=ys[:, j0:j0 + MMW],
                                     start=False, stop=True)
                rsl = R[:, jg * GW:(jg + 1) * GW]
                bdst = rsl if ic == 0 else bp.tile([D, GW], BF16, tag="bt")
                if gi in POOL_EVAC:
                    nc.gpsimd.tensor_scalar(bdst, ps[:], 0.0, None,
                                            op0=OP.add, op1=OP.max,
                                            accum_out=cm[:, gi:gi + 1])
                else:
                    nc.scalar.activation(bdst, ps[:], ACTF.Copy,
                                         bias=0.0, scale=1.0)
                    scr = scp.tile([D, GW], BF16, tag="scr")
                    nc.vector.tensor_scalar(scr[:], bdst, 0.0, None,
                                            op0=OP.add, op1=OP.max,
                                            accum_out=cm[:, gi:gi + 1])
                if ic > 0:
                    eng = nc.gpsimd if gi in ROW_POOL_GI else nc.vector
                    eng.tensor_tensor(rsl, bdst, rsl, op=OP.max)
            if ic == 2 and b + 1 < BPC:
                nxt = emit_load(b + 1)

        # row finish: max over partitions via PE transposes + grouped reduce
        rt = prt.tile([D, N], BF16, tag="rt")
        for t in range(NI):
            nc.tensor.transpose(rt[:, t * ICH:(t + 1) * ICH],
                                R[:, t * ICH:(t + 1) * ICH], ident[:])
        rr = small.tile([D, NI], F32, tag="rr")
        nc.vector.tensor_reduce(rr[:], rt[:].rearrange("p (t q) -> p t q", q=ICH),
                                axis=AX.X, op=OP.max)
        cmf = small.tile([D, NI], F32, tag="cmf")
        nc.vector.tensor_reduce(cmf[:], cm[:].rearrange("p (i g) -> p i g", g=NG),
                                axis=AX.X, op=OP.max)
        tX = small.tile([D, NI], F32, tag="tX")
        nc.vector.tensor_tensor(tX[:], rr[:], mcols[:], op=OP.mult)
        tY = small.tile([D, NI], F32, tag="tY")
        nc.vector.tensor_tensor(tY[:], cmf[:], mcols[:], op=OP.mult)
        sX = small.tile([D, 1], F32, tag="sX")
        nc.vector.tensor_reduce(sX[:], tX[:], axis=AX.X, op=OP.add)
        sY = small.tile([D, 1], F32, tag="sY")
        nc.vector.tensor_reduce(sY[:], tY[:], axis=AX.X, op=OP.add)
        nc.vector.tensor_tensor(acc[:, 0:1], acc[:, 0:1], sX[:], op=OP.add)
        nc.vector.tensor_tensor(acc[:, 1:2], acc[:, 1:2], sY[:], op=OP.add)
        if nxt is not None:
            st = nxt
            nxt = None

    nc.sync.dma_start(out=out_d, in_=acc[:])


def _hilo_e4m3(v):
    """Split v >= 0 into hi+lo fp8e4m3 (clipped to the 240 max)."""
    hi = np.minimum(v, 240.0).astype(ml_dtypes.float8_e4m3)
    lo = (v - hi.astype(np.float64)).astype(ml_dtypes.float8_e4m3)
    return hi, lo


def prepare_in_maps(x, y, mask):
    xb = np.asarray(x).astype(ml_dtypes.bfloat16)          # [B, D, N]
    yb = np.asarray(y).astype(ml_dtypes.bfloat16)
    mf = np.asarray(mask).astype(np.float64)               # [B, N]
    x2 = (xb.astype(np.float64) ** 2).sum(axis=1)          # [B, N]
    y2 = (yb.astype(np.float64) ** 2).sum(axis=1)
    vx = x2 / 2 + MBIG * (1.0 - mf)
    vy = y2 / 2 + MBIG * (1.0 - mf)
    xhi, xlo = _hilo_e4m3(vx)
    yhi, ylo = _hilo_e4m3(vy)
    pfl = np.empty((B, 2, 2, N), dtype=ml_dtypes.float8_e4m3)
    pfr = np.empty((B, 2, 2, N), dtype=ml_dtypes.float8_e4m3)
    pfl[:, 0, :, :] = -1.0
    pfl[:, 1, 0, :] = xhi
    pfl[:, 1, 1, :] = xlo
    pfr[:, 0, 0, :] = yhi
    pfr[:, 0, 1, :] = ylo
    pfr[:, 1, :, :] = -1.0
    mcols = np.ascontiguousarray(
        mf.astype(np.float32).reshape(B, NI, ICH).transpose(0, 2, 1))
    in_maps = []
    for c in range(CORES):
        s = slice(c * BPC, (c + 1) * BPC)
        in_maps.append({
            "x": np.ascontiguousarray(xb[s]),
            "y": np.ascontiguousarray(yb[s]),
            "pfl": np.ascontiguousarray(pfl[s]),
            "pfr": np.ascontiguousarray(pfr[s]),
            "mcols": np.ascontiguousarray(mcols[s]),
        })
    return in_maps


def finish(per_core_outs):
    """per_core_outs: list of 8 arrays [128, 2] -> scalar loss."""
    total = 0.0
    for o in per_core_outs:
        total += np.asarray(o, dtype=np.float64).sum()
    return np.float32(-2.0 * total / B)


_NC = None


def kernel(x, y, mask):
    global _NC
    if _NC is None:
        _NC = build_nc()
    from concourse.bass_utils import run_bass_kernel_spmd
    in_maps = prepare_in_maps(np.asarray(x), np.asarray(y), np.asarray(mask))
    res = run_bass_kernel_spmd(_NC, in_maps, list(range(CORES)))
    return finish([res.results[c]["out"] for c in range(CORES)])


# revision 12
# speedup vs baseline: 1.1479x; 1.0098x over previous
"""Chamfer loss Bass/Tile kernel for Trainium2 (8 NeuronCores, SPMD).

Problem: x, y [B=32, D=128, N=2048] f32, mask [B, N] bool (shared by x and y).
  d[b,i,j] = ||x_i - y_j||^2;  loss = mean_b( sum_j min_i d + sum_i min_j d )
  (mins/sums over valid entries only).

Strategy (data-parallel over batch, 4 batches per core):
  - Work in S = -d/2 = G - x2/2 - y2/2 form. PSUM group [128 x 1024] gets
    S directly: an fp8e4 DoubleRow "prefill" matmul (K=2, hi/lo split rows,
    0.5 cycles/col) injects -x2m/2 (per-partition) and -y2m/2 (per-column)
    including +BIG masking, then bf16 main matmuls accumulate G on top.
    No per-tile bias work remains for the vector engines.
  - Evac+col: ~70% of groups evacuate PSUM->bf16 SBUF via ACT (Copy) and
    run a DVE tensor_scalar (4x mode) whose accum_out gives max_j S per
    partition; ~30% of groups use a single Pool tensor_scalar that does
    evac + accum in one op. min_j d = -2 max_j S.
  - Row path: running elementwise max over the 16 i-chunks (TT max, split
    DVE/Pool); the ic==0 evac writes straight into R. Finished per batch by
    PE transposes + a grouped tensor_reduce -> max_i S per j.
  - Masked sums against host-shipped mask cols; host scales by -2/B.
"""

import numpy as np
import ml_dtypes
from contextlib import ExitStack

import concourse.mybir as mybir
import concourse.tile as tile
from concourse import bacc
from concourse.masks import make_identity

F32 = mybir.dt.float32
BF16 = mybir.dt.bfloat16
F8E4 = mybir.dt.float8e4
AX = mybir.AxisListType
OP = mybir.AluOpType
ACTF = mybir.ActivationFunctionType
DR = mybir.MatmulPerfMode.DoubleRow

B, D, N = 32, 128, 2048
CORES = 8
BPC = B // CORES          # batches per core
ICH, NI = 128, N // 128   # i-chunk size / count
GW, NG = 1024, N // 1024  # j-group width / count (evac granularity)
MMW = 512                 # matmul width (one PSUM bank)
MBIG = 288.0              # mask push (d shifted by 2*MBIG per masked side)

# groups (gi = ic*NG+jg) that evac via Pool tensor_scalar (fused col accum);
# the rest evac via ACT Copy + a DVE col pass.
POOL_EVAC = frozenset(gi for gi in range(NI * NG) if gi % 10 in (3, 6, 9))
# row TT ops (ic>=1): sent to Pool when gi % 7 == 2, else DVE.
ROW_POOL_GI = frozenset((2, 5, 8, 11, 15, 20, 24, 28, 31))


def build_nc():
    nc = bacc.Bacc("TRN2", target_bir_lowering=False, debug=False)
    x_d = nc.dram_tensor("x", [BPC, D, N], BF16, kind="ExternalInput").ap()
    y_d = nc.dram_tensor("y", [BPC, D, N], BF16, kind="ExternalInput").ap()
    pfl_d = nc.dram_tensor("pfl", [BPC, 2, 2, N], F8E4, kind="ExternalInput").ap()
    pfr_d = nc.dram_tensor("pfr", [BPC, 2, 2, N], F8E4, kind="ExternalInput").ap()
    mcols_d = nc.dram_tensor("mcols", [BPC, D, NI], F32, kind="ExternalInput").ap()
    out_d = nc.dram_tensor("out", [D, 2], F32, kind="ExternalOutput").ap()

    with tile.TileContext(nc) as tc:
        with ExitStack() as ctx:
            _emit(ctx, tc, out_d, x_d, y_d, pfl_d, pfr_d, mcols_d)
    nc.compile()
    return nc


def _emit(ctx, tc, out_d, x_d, y_d, pfl_d, pfr_d, mcols_d):
    nc = tc.nc
    io = ctx.enter_context(tc.tile_pool(name="io", bufs=2))
    pf = ctx.enter_context(tc.tile_pool(name="pf", bufs=2))
    bp = ctx.enter_context(tc.tile_pool(name="bp", bufs=6))
    rp = ctx.enter_context(tc.tile_pool(name="rp", bufs=2))
    scp = ctx.enter_context(tc.tile_pool(name="scp", bufs=2))
    small = ctx.enter_context(tc.tile_pool(name="small", bufs=2))
    accp = ctx.enter_context(tc.tile_pool(name="accp", bufs=1))
    pre = ctx.enter_context(tc.tile_pool(name="pre", bufs=1))
    pp = ctx.enter_context(tc.tile_pool(name="pp", bufs=3, space="PSUM"))
    prt = ctx.enter_context(tc.tile_pool(name="prt", bufs=1, space="PSUM"))

    acc = accp.tile([D, 2], F32)
    nc.vector.memset(acc[:], 0.0)
    ident = pre.tile([ICH, ICH], BF16, tag="ident")
    make_identity(nc, ident[:])

    def emit_load(b):
        st = {}
        st["xs"] = io.tile([D, N], BF16, tag="xs", name=f"xs{b}")
        st["ys"] = io.tile([D, N], BF16, tag="ys", name=f"ys{b}")
        st["pfl"] = pf.tile([2, 2, N], F8E4, tag="pfl", name=f"pfl{b}")
        st["pfr"] = pf.tile([2, 2, N], F8E4, tag="pfr", name=f"pfr{b}")
        st["mcols"] = small.tile([D, NI], F32, tag="mcols", name=f"mcols{b}")
        nc.sync.dma_start(out=st["pfl"][:], in_=pfl_d[b])
        nc.sync.dma_start(out=st["pfr"][:], in_=pfr_d[b])
        nc.sync.dma_start(out=st["mcols"][:], in_=mcols_d[b])
        nc.sync.dma_start(out=st["xs"][:], in_=x_d[b])
        nc.sync.dma_start(out=st["ys"][:], in_=y_d[b])
        return st

    st = emit_load(0)
    nxt = None
    for b in range(BPC):
        xs, ys, pfl, pfr, mcols = (st["xs"], st["ys"], st["pfl"], st["pfr"],
                                   st["mcols"])
        R = rp.tile([D, N], BF16, tag="R", name=f"R{b}")
        cm = small.tile([D, NI * NG], F32, tag="cm", name=f"cm{b}")
        nc.vector.memset(cm[:], -1e30)
        for ic in range(NI):
            lsl = slice(ic * ICH, (ic + 1) * ICH)
            for jg in range(NG):
                gi = ic * NG + jg
                ps = pp.tile([D, GW], F32, tag="ps")
                for h in range(GW // MMW):
                    j0 = jg * GW + h * MMW
                    psl = ps[:, h * MMW:(h + 1) * MMW]
                    nc.tensor.matmul(psl, lhsT=pfl[:, :, lsl],
                                     rhs=pfr[:, :, j0:j0 + MMW],
                                     start=True, stop=False, perf_mode=DR)
                    nc.tensor.matmul(psl, lhsT=xs[:, lsl],
                                     rhs=ys[:, j0:j0 + MMW],
                                     start=False, stop=True)
                rsl = R[:, jg * GW:(jg + 1) * GW]
                bdst = rsl if ic == 0 else bp.tile([D, GW], BF16, tag="bt")
                if gi in POOL_EVAC:
                    nc.gpsimd.tensor_scalar(bdst, ps[:], 0.0, None,
                                            op0=OP.add, op1=OP.max,
                                            accum_out=cm[:, gi:gi + 1])
                else:
                    nc.scalar.activation(bdst, ps[:], ACTF.Copy,
                                         bias=0.0, scale=1.0)
                    scr = scp.tile([D, GW], BF16, tag="scr")
                    nc.vector.tensor_scalar(scr[:], bdst, 0.0, None,
                                            op0=OP.add, op1=OP.max,
                                            accum_out=cm[:, gi:gi + 1])
                if ic > 0:
                    eng = nc.gpsimd if gi in ROW_POOL_GI else nc.vector
                    eng.tensor_tensor(rsl, bdst, rsl, op=OP.max)
            if ic == 2 and b + 1 < BPC:
                nxt = emit_load(b + 1)

        # row finish: max over partitions via PE transposes + grouped reduce
        rt = prt.tile([D, N], BF16, tag="rt")
        for t in range(NI):
            nc.tensor.transpose(rt[:, t * ICH:(t + 1) * ICH],
                                R[:, t * ICH:(t + 1) * ICH], ident[:])
        rr = small.tile([D, NI], F32, tag="rr")
        nc.vector.tensor_reduce(rr[:], rt[:].rearrange("p (t q) -> p t q", q=ICH),
                                axis=AX.X, op=OP.max)
        cmf = small.tile([D, NI], F32, tag="cmf")
        nc.vector.tensor_reduce(cmf[:], cm[:].rearrange("p (i g) -> p i g", g=NG),
                                axis=AX.X, op=OP.max)
        tX = small.tile([D, NI], F32, tag="tX")
        nc.vector.tensor_tensor(tX[:], rr[:], mcols[:], op=OP.mult)
        tY = small.tile([D, NI], F32, tag="tY")
        nc.vector.tensor_tensor(tY[:], cmf[:], mcols[:], op=OP.mult)
        sX = small.tile([D, 1], F32, tag="sX")
        nc.vector.tensor_reduce(sX[:], tX[:], axis=AX.X, op=OP.add)
        sY = small.tile([D, 1], F32, tag="sY")
        nc.vector.tensor_reduce(sY[:], tY[:], axis=AX.X, op=OP.add)
        nc.vector.tensor_tensor(acc[:, 0:1], acc[:, 0:1], sX[:], op=OP.add)
        nc.vector.tensor_tensor(acc[:, 1:2], acc[:, 1:2], sY[:], op=OP.add)
        if nxt is not None:
            st = nxt
            nxt = None

    nc.sync.dma_start(out=out_d, in_=acc[:])


def _hilo_e4m3(v):
    """Split v >= 0 into hi+lo fp8e4m3 (clipped to the 240 max)."""
    hi = np.minimum(v, 240.0).astype(ml_dtypes.float8_e4m3)
    lo = (v - hi.astype(np.float64)).astype(ml_dtypes.float8_e4m3)
    return hi, lo


def prepare_in_maps(x, y, mask):
    xb = np.asarray(x).astype(ml_dtypes.bfloat16)          # [B, D, N]
    yb = np.asarray(y).astype(ml_dtypes.bfloat16)
    mf = np.asarray(mask).astype(np.float64)               # [B, N]
    x2 = (xb.astype(np.float64) ** 2).sum(axis=1)          # [B, N]
    y2 = (yb.astype(np.float64) ** 2).sum(axis=1)
    vx = x2 / 2 + MBIG * (1.0 - mf)
    vy = y2 / 2 + MBIG * (1.0 - mf)
    xhi, xlo = _hilo_e4m3(vx)
    yhi, ylo = _hilo_e4m3(vy)
    pfl = np.empty((B, 2, 2, N), dtype=ml_dtypes.float8_e4m3)
    pfr = np.empty((B, 2, 2, N), dtype=ml_dtypes.float8_e4m3)
    pfl[:, 0, :, :] = -1.0
    pfl[:, 1, 0, :] = xhi
    pfl[:, 1, 1, :] = xlo
    pfr[:, 0, 0, :] = yhi
    pfr[:, 0, 1, :] = ylo
    pfr[:, 1, :, :] = -1.0
    mcols = np.ascontiguousarray(
        mf.astype(np.float32).reshape(B, NI, ICH).transpose(0, 2, 1))
    in_maps = []
    for c in range(CORES):
        s = slice(c * BPC, (c + 1) * BPC)
        in_maps.append({
            "x": np.ascontiguousarray(xb[s]),
            "y": np.ascontiguousarray(yb[s]),
            "pfl": np.ascontiguousarray(pfl[s]),
            "pfr": np.ascontiguousarray(pfr[s]),
            "mcols": np.ascontiguousarray(mcols[s]),
        })
    return in_maps


def finish(per_core_outs):
    """per_core_outs: list of 8 arrays [128, 2] -> scalar loss."""
    total = 0.0
    for o in per_core_outs:
        total += np.asarray(o, dtype=np.float64).sum()
    return np.float32(-2.0 * total / B)


_NC = None


def kernel(x, y, mask):
    global _NC
    if _NC is None:
        _NC = build_nc()
    from concourse.bass_utils import run_bass_kernel_spmd
    in_maps = prepare_in_maps(np.asarray(x), np.asarray(y), np.asarray(mask))
    res = run_bass_kernel_spmd(_NC, in_maps, list(range(CORES)))
    return finish([res.results[c]["out"] for c in range(CORES)])
